# revision 10
# baseline (speedup 1.0000x reference)
"""Self-contained Trainium2 Bass kernel for a 2-layer GCN encoder
(PyG GCNConv x2 with LeakyReLU), distributed over 8 NeuronCores.

kernel(**inputs) takes the full unsharded inputs (X [50000,512] f32,
edge_index [2,800000] int64, W1/b1/W2/b2) and returns the full
[50000,128] f32 output.

Structure (v2):
- nodes sharded round-robin by 128-block across 8 cores; each core owns
  bpc=50 blocks, grouped into S=5 position segments of 10 blocks.
- dense phase g = dinv * (X @ W) computed per segment, AllGathered per
  segment (5 small AGs, Shared-output) so message passing can start as
  soon as segment 0 lands and the remaining AGs overlap compute.
- message passing is pass-major: pass s processes ALL dst blocks' edges
  whose source lies in segment s. The running per-block sum is kept in
  SBUF (bf16) and re-injected into PSUM at the start of the next pass
  via an identity matmul; the pass-0 injection doubles as the GCN
  self-loop term. Edges are stored as one continuous 128-slot chunk
  stream per (core, segment) (cross-core max padded per block, so the
  chunk->block covering structure is SPMD-uniform); chunks straddling a
  block boundary are matmul'd twice with disjoint one-hot masks.
- the leaky-relu + dinv scale sink is a single Prelu activation on the
  (otherwise idle) scalar engine.
- layer 2 reuses the exact same gather index stream / one-hot metadata
  (same graph); its dense phase is interleaved segment-by-segment into
  layer 1's final pass so AG2 overlaps mp1's tail.
"""

import sys
if "/opt/trn_rl_repo" not in sys.path:
    sys.path.insert(0, "/opt/trn_rl_repo")

import math
from dataclasses import dataclass, field

import numpy as np
import ml_dtypes

import concourse.bass as bass
import concourse.tile as tile
from concourse import bacc, mybir
from concourse.bass_utils import run_bass_kernel_spmd

FP32 = mybir.dt.float32
BF16 = mybir.dt.bfloat16
I32 = mybir.dt.int32
I16 = mybir.dt.int16


@dataclass
class Cfg:
    n: int          # real node count
    e: int          # real edge count
    d_in: int
    h1: int
    h2: int
    cores: int = 8
    bpc: int = 50   # 128-node dst blocks per core
    segs: int = 5   # position segments (AllGather granularity)
    neg: float = 0.2
    gbufs: int = 10  # gather pool buffers
    shared_ag: bool = False

    @property
    def npad(self):
        return self.cores * self.bpc * 128

    @property
    def shard(self):
        return self.bpc * 128

    @property
    def bps(self):
        return self.bpc // self.segs     # blocks per (core, segment)

    @property
    def segrows(self):
        return self.cores * self.bps * 128   # rows per seg table


@dataclass
class Meta:
    # per seg: number of 128-slot chunks
    nch: list = field(default_factory=list)
    # per seg: list of dma_gather calls (col0_in_idx_tile, nck)
    calls: list = field(default_factory=list)
    # per seg: covering list [(block_p, chunk_idx, global_dst_col)]
    covs: list = field(default_factory=list)
    ncov: int = 0
    idx_cols: int = 0
    bias1_nz: bool = False
    bias2_nz: bool = False


def preprocess(cfg: Cfg, X, edge_index, W1, b1, W2, b2):
    n, npad = cfg.n, cfg.npad
    C, S, BPC, BPS = cfg.cores, cfg.segs, cfg.bpc, cfg.bps
    nblk = npad // 128

    src = np.asarray(edge_index[0], dtype=np.int64)
    dst = np.asarray(edge_index[1], dtype=np.int64)
    E = src.size

    deg = np.bincount(dst, minlength=npad).astype(np.float32) + 1.0
    dinv = (1.0 / np.sqrt(deg)).astype(np.float32)

    # node -> (core, p, lane); segment s = p // BPS
    ids = np.arange(npad, dtype=np.int64)
    nb = ids >> 7
    lane_n = ids & 127
    core_of = nb % C
    p_of = nb // C
    q_of = p_of % BPS
    segrow = core_of * (BPS * 128) + q_of * 128 + lane_n  # row in seg table
    seg_of = p_of // BPS

    e_seg = seg_of[src]
    e_row = segrow[src]
    e_core = core_of[dst]
    e_p = p_of[dst]
    e_lane = (dst & 127)

    key = (e_core * S + e_seg) * BPC + e_p
    cnt = np.bincount(key, minlength=C * S * BPC).reshape(C, S, BPC)
    maxcnt = cnt.max(axis=0)                      # [S, BPC]

    # chunk layout per seg (shared across cores)
    starts = np.zeros((S, BPC + 1), np.int64)
    np.cumsum(maxcnt, axis=1, out=starts[:, 1:])
    L = starts[:, -1]                              # stream length per seg
    nch = [int(math.ceil(int(L[s]) / 128)) for s in range(S)]
    Lpad = [nch[s] * 128 for s in range(S)]

    meta = Meta(
        nch=nch,
        bias1_nz=bool(np.any(np.asarray(b1) != 0)),
        bias2_nz=bool(np.any(np.asarray(b2) != 0)),
    )

    # covering structure per seg: block-major (p asc, chunk asc)
    col = 0
    for s in range(S):
        covs = []
        ends = starts[s].copy()
        ends[-1] = Lpad[s]                         # tail slots -> last block
        for p in range(BPC):
            c0 = int(starts[s, p]) // 128
            c1 = (int(ends[p + 1]) - 1) // 128
            for c in range(c0, c1 + 1):
                covs.append((p, c, col))
                col += 1
        meta.covs.append(covs)
    meta.ncov = col

    # gather call structure per seg: calls of up to 8 chunks
    colbase = 0
    for s in range(S):
        calls = []
        for off in range(0, nch[s], 8):
            nck = min(8, nch[s] - off)
            calls.append((colbase + off * 8, nck))
        meta.calls.append(calls)
        colbase += Lpad[s] // 16
    meta.idx_cols = colbase

    # per-core edge placement
    order = np.lexsort((e_row, e_p, e_seg, e_core))
    okey = key[order]
    # position within (core,seg,p) group
    gstart = np.zeros(C * S * BPC + 1, np.int64)
    np.cumsum(cnt.reshape(-1), out=gstart[1:])
    pos_in_grp = np.arange(E, dtype=np.int64) - gstart[okey]
    # global slot within the (core, seg) stream
    slot = starts[(okey // BPC) % S, okey % BPC] + pos_in_grp

    o_core = okey // (S * BPC)
    o_seg = (okey // BPC) % S
    o_row = e_row[order]
    o_lane = e_lane[order]

    idx_arr = [np.zeros((C, Lpad[s]), np.int16) for s in range(S)]
    dstl = [np.full((C, Lpad[s]), -1, np.int32) for s in range(S)]
    for s in range(S):
        m = o_seg == s
        idx_arr[s][o_core[m], slot[m]] = o_row[m].astype(np.int16)
        dstl[s][o_core[m], slot[m]] = o_lane[m].astype(np.int32)

    # slot -> owning block map per seg (shared)
    sblk = []
    for s in range(S):
        sb = np.zeros(Lpad[s], np.int64)
        ends = starts[s].copy()
        ends[-1] = Lpad[s]
        for p in range(BPC):
            sb[int(starts[s, p]):int(ends[p + 1])] = p
        sblk.append(sb)

    # replicated tensors
    XT = np.zeros((cfg.d_in, npad), np.float32)
    XT[:, :n] = np.asarray(X, np.float32).T
    XT = XT.astype(ml_dtypes.bfloat16)
    W1b = np.asarray(W1, np.float32).astype(ml_dtypes.bfloat16)
    W2b = np.asarray(W2, np.float32).astype(ml_dtypes.bfloat16)
    iota4 = np.ascontiguousarray(np.broadcast_to(
        np.arange(128, dtype=np.float32)[None, None, :],
        (128, 8, 128))).astype(ml_dtypes.bfloat16)
    ident = np.eye(128, dtype=np.float32).astype(ml_dtypes.bfloat16)

    in_maps = []
    for c in range(C):
        # idx tile: concat per-seg streams, 16-partition wrap, tiled to 128
        flat = np.concatenate([idx_arr[s][c] for s in range(S)])
        assert flat.size == meta.idx_cols * 16
        idx_tile = np.ascontiguousarray(
            np.tile(flat.reshape(-1, 16).T, (8, 1)))       # [128, idx_cols]

        # dst one-hot lane columns, one per covering, in cov order
        cols = np.full((meta.ncov, 128), -1, np.int32)
        for s in range(S):
            dl = dstl[s][c]
            sb = sblk[s]
            for (p, ch, col_i) in meta.covs[s]:
                sl = slice(ch * 128, (ch + 1) * 128)
                cols[col_i] = np.where(sb[sl] == p, dl[sl], -1)
        dst_tile = np.ascontiguousarray(cols.T.astype(np.float32)
                                        ).astype(ml_dtypes.bfloat16)

        node_sel = ((np.arange(BPC)[:, None] * C + c) * 128
                    + np.arange(128)[None, :]).reshape(-1)
        dv = dinv[node_sel].reshape(BPC, 128).T            # [128, BPC]
        m = {
            "xt": np.ascontiguousarray(XT[:, node_sel]),
            "w1": W1b, "w2": W2b,
            "idx": idx_tile,
            "dstloc": dst_tile,
            "dinv": np.ascontiguousarray(dv).astype(np.float32),
            "iota4": iota4,
            "ident": ident,
        }
        in_maps.append(m)
    assert not meta.bias1_nz and not meta.bias2_nz, \
        "nonzero GCN biases not supported by this kernel variant"
    return in_maps, meta


def build(cfg: Cfg, meta: Meta, stop_after: str = 'full'):
    nc = bacc.Bacc("TRN2", target_bir_lowering=False, debug=False,
                   num_devices=cfg.cores, num_swdge_queues=4)
    C, S, BPC, BPS = cfg.cores, cfg.segs, cfg.bpc, cfg.bps
    kin, kh1 = cfg.d_in // 128, cfg.h1 // 128
    segrows = cfg.segrows
    AT = mybir.ActivationFunctionType
    OP = mybir.AluOpType
    aspace = "Shared" if cfg.shared_ag else "Local"

    xt = nc.dram_tensor("xt", [cfg.d_in, cfg.shard], BF16, kind="ExternalInput")
    w1 = nc.dram_tensor("w1", [cfg.d_in, cfg.h1], BF16, kind="ExternalInput")
    w2 = nc.dram_tensor("w2", [cfg.h1, cfg.h2], BF16, kind="ExternalInput")
    idx = nc.dram_tensor("idx", [128, meta.idx_cols], I16, kind="ExternalInput")
    dstloc = nc.dram_tensor("dstloc", [128, meta.ncov], BF16, kind="ExternalInput")
    dinv = nc.dram_tensor("dinv", [128, BPC], FP32, kind="ExternalInput")
    iota_d = nc.dram_tensor("iota4", [128, 8, 128], BF16, kind="ExternalInput")
    ident_d = nc.dram_tensor("ident", [128, 128], BF16, kind="ExternalInput")
    out = nc.dram_tensor("out", [cfg.shard, cfg.h2], FP32, kind="ExternalOutput")

    rg = [list(range(C))]
    stop = stop_after

    with tile.TileContext(nc) as tc:
        with (
            tc.tile_pool(name="constp", bufs=1) as constp,
            tc.tile_pool(name="persist", bufs=1) as persist,
            tc.tile_pool(name="dram", bufs=1, space="DRAM") as dram,
            tc.tile_pool(name="ohp", bufs=8) as ohp,
            tc.tile_pool(name="sp", bufs=6) as sp,
            tc.tile_pool(name="pp", bufs=6, space="PSUM") as pp,
        ):
            g1s = [dram.tile([BPS * 128, cfg.h1], BF16, name=f"g1s{s}")
                   for s in range(S)]
            g1f = [dram.tile([segrows, cfg.h1], BF16, name=f"g1f{s}",
                             addr_space=aspace) for s in range(S)]
            z1d = [dram.tile([BPS * 128, cfg.h1], BF16, name=f"z1d{s}")
                   for s in range(S)]
            g2s = [dram.tile([BPS * 128, cfg.h2], BF16, name=f"g2s{s}")
                   for s in range(S)]
            g2f = [dram.tile([segrows, cfg.h2], BF16, name=f"g2f{s}",
                             addr_space=aspace) for s in range(S)]

            # ---- constants ----
            w1sb = constp.tile([128, kin, cfg.h1], BF16)
            for k in range(kin):
                nc.sync.dma_start(w1sb[:, k, :], w1[k * 128:(k + 1) * 128, :])
            w2sb = constp.tile([128, kh1, cfg.h2], BF16)
            for k in range(kh1):
                nc.sync.dma_start(w2sb[:, k, :], w2[k * 128:(k + 1) * 128, :])
            idxsb = constp.tile([128, meta.idx_cols], I16)
            nc.sync.dma_start(idxsb[:], idx[:])
            dstsb = constp.tile([128, meta.ncov], BF16)
            nc.sync.dma_start(dstsb[:], dstloc[:])
            dvsb = constp.tile([128, BPC], FP32)
            nc.sync.dma_start(dvsb[:], dinv[:])
            iotasb = constp.tile([128, 8, 128], BF16)
            nc.sync.dma_start(iotasb[:], iota_d[:])
            identsb = constp.tile([128, 128], BF16)
            nc.sync.dma_start(identsb[:], ident_d[:])

            # persistent SBUF tensors
            g1own = persist.tile([128, BPC, cfg.h1], BF16)
            acc1 = persist.tile([128, BPC, cfg.h1], BF16)
            g2own = persist.tile([128, BPC, cfg.h2], BF16)
            acc2 = persist.tile([128, BPC, cfg.h2], BF16)

            # ---- dense layer 1, per segment, AG per segment ----
            with tc.tile_pool(name="xtp", bufs=1) as xtp:
                xts = xtp.tile([128, kin, cfg.shard], BF16)
                for k in range(kin):
                    nc.sync.dma_start(xts[:, k, :], xt[k * 128:(k + 1) * 128, :])
                for s in range(S):
                    for q in range(BPS):
                        p = s * BPS + q
                        ps = pp.tile([128, 256], FP32, tag="ps")
                        for k in range(kin):
                            nc.tensor.matmul(
                                ps[:], xts[:, k, p * 128:(p + 1) * 128],
                                w1sb[:, k, :],
                                start=(k == 0), stop=(k == kin - 1))
                        nc.scalar.mul(g1own[:, p, :], ps[:], dvsb[:, p:p + 1])
                        nc.sync.dma_start(g1s[s][q * 128:(q + 1) * 128, :],
                                          g1own[:, p, :])
                    if stop != "p1":
                        nc.gpsimd.collective_compute(
                            "AllGather", OP.bypass, replica_groups=rg,
                            ins=[g1s[s].opt()], outs=[g1f[s].opt()])

            if stop in ("p1", "ag1"):
                nc.compile()
                return nc

            qctr = [0]

            def gather_pass(s, gf, h, gp):
                tiles = []
                for (col0, nck) in meta.calls[s]:
                    g = gp.tile([128, 8, h], BF16, tag=f"g{h}", name="g")
                    nc.gpsimd.dma_gather(
                        g[:, 0:nck, :], gf[s][:, :],
                        idxsb[:, col0:col0 + nck * 8],
                        nck * 128, nck * 128, h,
                        queue_num=(s % 2) * 2 + qctr[0] % 2)
                    qctr[0] += 1
                    tiles.append(g)
                return tiles

            def msg_pass(s, tiles, h, accsrc, acc, last, sink, interleave=None):
                """One message-passing pass over all blocks for segment s."""
                covs = meta.covs[s]
                ncov_s = len(covs)
                colbase = covs[0][2]
                # lazily-built one-hot batches (8 coverings each)
                oh_tiles = [None] * ((ncov_s + 7) // 8)

                def get_oh(j):
                    bi = j // 8
                    if oh_tiles[bi] is None:
                        j0 = bi * 8
                        nb = min(8, ncov_s - j0)
                        oh = ohp.tile([128, 8, 128], BF16, tag="oh")
                        dcol = dstsb[:, colbase + j0:colbase + j0 + nb]
                        nc.vector.tensor_tensor(
                            oh[:, 0:nb, :], iotasb[:, 0:nb, :],
                            dcol.broadcast_to([128, nb, 128]),
                            op=OP.is_equal)
                        oh_tiles[bi] = oh
                    return oh_tiles[bi][:, j % 8, :]

                ji = 0
                for p in range(BPC):
                    psf = pp.tile([128, 256], FP32, tag="ps", name="psf")
                    ps = psf[:, 0:h]
                    has_covs = ji < ncov_s and covs[ji][0] == p
                    # inject running sum (or self-loop term for pass 0)
                    nc.tensor.matmul(ps[:], identsb[:], accsrc[:, p, :],
                                     start=True, stop=not has_covs)
                    while ji < ncov_s and covs[ji][0] == p:
                        (_, ch, _col) = covs[ji]
                        mt = tiles[ch // 8][:, ch % 8, :]
                        nc.tensor.matmul(
                            ps[:], get_oh(ji), mt,
                            start=False,
                            stop=(ji == ncov_s - 1 or covs[ji + 1][0] != p))
                        ji += 1
                    if not last:
                        nc.scalar.copy(acc[:, p, :], ps[:])
                    else:
                        sink(p, ps)
                    if interleave is not None and p % BPS == BPS - 1:
                        interleave(p // BPS)

            # ---- layer 1 message passing ----
            def z1_sink(p, ps):
                zt = sp.tile([128, cfg.h1], BF16, tag="z1")
                nc.scalar.activation(zt[:], ps[:], AT.Prelu, bias=0.0,
                                     scale=dvsb[:, p:p + 1], alpha=cfg.neg)
                s, q = p // BPS, p % BPS
                nc.sync.dma_start(z1d[s][q * 128:(q + 1) * 128, :], zt[:])

            # dense layer 2 for one segment (interleaved into mp1 last pass)
            def dense2_seg(s2):
                with tc.tile_pool(name=f"ztp{s2}", bufs=1) as ztp:
                    z1t = ztp.tile([128, kh1, BPS * 128], BF16, tag="z1t")
                    for k in range(kh1):
                        nc.sync.dma_start_transpose(
                            out=z1t[:, k, :],
                            in_=z1d[s2][:, k * 128:(k + 1) * 128])
                    for q in range(BPS):
                        p = s2 * BPS + q
                        psd = pp.tile([128, 256], FP32, tag="ps", name="psd")
                        ps = psd[:, 0:cfg.h2]
                        for k in range(kh1):
                            nc.tensor.matmul(
                                ps[:], z1t[:, k, q * 128:(q + 1) * 128],
                                w2sb[:, k, :],
                                start=(k == 0), stop=(k == kh1 - 1))
                        nc.scalar.mul(g2own[:, p, :], ps[:], dvsb[:, p:p + 1])
                        nc.sync.dma_start(g2s[s2][q * 128:(q + 1) * 128, :],
                                          g2own[:, p, :])
                if stop != "p4":
                    nc.gpsimd.collective_compute(
                        "AllGather", OP.bypass, replica_groups=rg,
                        ins=[g2s[s2].opt()], outs=[g2f[s2].opt()])

            with tc.tile_pool(name="gp1", bufs=cfg.gbufs) as gp1:
                for s in range(S):
                    tiles = gather_pass(s, g1f, cfg.h1, gp1)
                    last = s == S - 1
                    msg_pass(s, tiles, cfg.h1,
                             accsrc=(g1own if s == 0 else acc1), acc=acc1,
                             last=last, sink=z1_sink,
                             interleave=(dense2_seg if last and stop not in
                                         ("p3",) else None))

            if stop in ("p3", "p4", "ag2"):
                nc.compile()
                return nc

            # ---- layer 2 message passing ----
            def out_sink(p, ps):
                ot = sp.tile([128, cfg.h2], FP32, tag="zo")
                nc.scalar.activation(ot[:], ps[:], AT.Prelu, bias=0.0,
                                     scale=dvsb[:, p:p + 1], alpha=cfg.neg)
                nc.sync.dma_start(out[p * 128:(p + 1) * 128, :], ot[:])

            with tc.tile_pool(name="gp2", bufs=cfg.gbufs) as gp2:
                for s in range(S):
                    tiles = gather_pass(s, g2f, cfg.h2, gp2)
                    msg_pass(s, tiles, cfg.h2,
                             accsrc=(g2own if s == 0 else acc2), acc=acc2,
                             last=(s == S - 1), sink=out_sink)

    nc.compile()
    return nc


def install_ntff_hook():
    """The agent image's antenv lacks axon_hooks; graft it so trace=True
    can reach the libaxon_pjrt NTFF profiling C ABI."""
    import sys as _sys, types as _types
    if "antenv.axon_hooks" in _sys.modules:
        return
    _sys.path.insert(0, "/root/.axon_site")
    from trn_agent_boot.trn_boot import _ntff_profile_via_ctypes
    hook = _ntff_profile_via_ctypes("/opt/axon/libaxon_pjrt.so")
    mod = _types.ModuleType("antenv.axon_hooks")
    mod._hook = hook
    mod.get_axon_ntff_profile_hook = lambda: mod._hook
    mod.set_axon_ntff_profile_hook = lambda h: setattr(mod, "_hook", h)
    _sys.modules["antenv.axon_hooks"] = mod
    import antenv
    antenv.axon_hooks = mod


def run(cfg: Cfg, X, edge_index, W1, b1, W2, b2, trace=False,
        stop_after='full', trace_cores=None):
    if trace:
        install_ntff_hook()
    import time
    t0 = time.time()
    in_maps, meta = preprocess(cfg, X, edge_index, W1, b1, W2, b2)
    t1 = time.time()
    nc = build(cfg, meta, stop_after=stop_after)
    t2 = time.time()
    print(f"preprocess {t1-t0:.1f}s, build+compile {t2-t1:.1f}s", flush=True)
    res = run_bass_kernel_spmd(nc, in_maps, core_ids=list(range(cfg.cores)),
                               trace=trace, trace_cores=trace_cores)
    print(f"hw run {time.time()-t2:.1f}s", flush=True)
    nblk = cfg.npad // 128
    full = np.empty((cfg.npad, cfg.h2), np.float32)
    for c in range(cfg.cores):
        o = res.results[c]["out"]
        for p, b in enumerate(range(c, nblk, cfg.cores)):
            full[b * 128:(b + 1) * 128] = o[p * 128:(p + 1) * 128]
    full = full[:cfg.n]
    return full, res, nc, in_maps, meta


_CFG = Cfg(n=50000, e=800000, d_in=512, h1=256, h2=128, cores=8)


def kernel(X, edge_index, W1, b1, W2, b2):
    full, _res, _nc, _maps, _meta = run(
        _CFG, X, edge_index, W1, b1, W2, b2, trace=False)
    return full
